# revision 15
# baseline (speedup 1.0000x reference)
"""Fully-connected GNN message-passing kernel for Trainium2 (8 NeuronCores).

Strategy (v2)
-------------
The reference graph is fully connected (each graph: all ordered pairs i != j),
so gather/segment_sum become dense per-graph math:

  edge-MLP layer 1:  concat([x[i], x[j]]) @ we1 == a_i + b_j
      with a = x @ we1[:H] + be1, b = x @ we1[H:]   (tiny matmuls)
  messages for ALL i,j pairs (incl. diagonal) are computed densely;
  agg_i = sum_j silu(silu(a_i+b_j) @ we2 + be2) - diagonal_term_i

Sharding: data-parallel over graphs, 2 graphs per core, weights replicated.
Feature-major layouts on-chip: features on partitions, nodes/edges on free dim.

The ACT (scalar) engine is the roofline: 2 SiLU passes over every edge
column at 1 col/cycle @1.2GHz.  v2 minimizes ACT instruction-overhead and
strips all other work off the ACT critical path:

  DVE+GPSIMD - build z1 = a_i + b_j as broadcast adds directly in SBUF
               (was: PE identity-matmuls into PSUM)
  ACT        - silu1 as one 8192-col instruction per chunk (SBUF->SBUF);
               silu2 as 2048-col instructions (PSUM->SBUF)
  PE         - mm2 (edge MLP2), node MLP, embeddings
  DVE        - per-receiver reductions (bf16, 2x mode), residuals
  GPSIMD     - z1-build share, diagonal extraction
"""

import numpy as np

# problem shapes (hardcoded per contract)
BS, N, IN_NF, H, EH, OUT_NF, L = 16, 128, 64, 256, 128, 64, 4
NCORES = 8
GPC = BS // NCORES            # graphs per core
NODES = GPC * N               # nodes per core
HC = H // 128                 # H partition chunks

# tuning knobs
CHUNK_RECV = 128              # receivers per z1 chunk (silu1 instruction)
CHUNK = CHUNK_RECV * N        # chunk columns (16384)
NCHUNK = N // CHUNK_RECV      # chunks per graph (1)
SLAB_RECV = 16                # receivers per mm2/silu2 slab
SLAB = SLAB_RECV * N          # slab columns (2048)
NSLAB = CHUNK // SLAB         # slabs per chunk (8)
ZPIECES = (2, 2, 2, 2)        # zbuild/silu1 piece sizes (slabs of 16 recv)
ZENG = ("dve", "dve", "gps", "dve")   # engine per zbuild piece
MMQ = 512                     # matmul moving-dim slice

MM_DT = "bf16"

_CACHE = {}


def _silu_np(x):
    return x / (1.0 + np.exp(-x))


def _canonical_edges():
    r = np.repeat(np.arange(N), N)
    c = np.tile(np.arange(N), N)
    m = r != c
    r, c = r[m], c[m]
    off = (np.arange(BS) * N)[:, None]
    rows = (r[None, :] + off).reshape(-1)
    cols = (c[None, :] + off).reshape(-1)
    return rows, cols


def _edges_match(rows, cols):
    """True if (rows, cols) describe the canonical fully-connected batch."""
    er, ec = _canonical_edges()
    rows = np.asarray(rows).astype(np.int64).ravel()
    cols = np.asarray(cols).astype(np.int64).ravel()
    if rows.shape != er.shape or cols.shape != ec.shape:
        return False
    if np.array_equal(rows, er) and np.array_equal(cols, ec):
        return True
    k1 = np.sort(rows * (BS * N) + cols)
    k2 = np.sort(er * (BS * N) + ec)
    return np.array_equal(k1, k2)


def _numpy_reference(h, rows, cols, w_in, b_in, w_out, b_out,
                     we1, be1, we2, be2, wn1, bn1, wn2, bn2):
    """Exact fallback (only used if inputs are not the canonical FC batch)."""
    f = np.float32
    x = h.reshape(BS * N, -1).astype(f) @ w_in.astype(f) + b_in.astype(f)
    rows = np.asarray(rows).astype(np.int64)
    cols = np.asarray(cols).astype(np.int64)
    for l in range(L):
        m = np.concatenate([x[rows], x[cols]], axis=-1)
        m = _silu_np(m @ we1[l].astype(f) + be1[l].astype(f))
        m = _silu_np(m @ we2[l].astype(f) + be2[l].astype(f))
        agg = np.zeros((BS * N, m.shape[-1]), f)
        np.add.at(agg, rows, m)
        u = np.concatenate([x, agg], axis=-1)
        u = _silu_np(u @ wn1[l].astype(f) + bn1[l].astype(f))
        u = u @ wn2[l].astype(f) + bn2[l].astype(f)
        x = x + u
    return x @ w_out.astype(f) + b_out.astype(f)


def _split_excess_waits(nc, mybir, cap=1):
    """The walrus build in this environment accepts only one sync-wait per
    instruction; move extra waits onto preceding same-engine NOPs."""
    n_split = 0
    for fn in nc.m.functions:
        for blk in fn.blocks:
            il = blk.instructions
            new = []
            changed = False
            for ins in il:
                si = ins.sync_info
                if si is not None and si.on_wait and len(si.on_wait) > cap:
                    waits = list(si.on_wait)
                    keep, extra = waits[-cap:], waits[:-cap]
                    for w in extra:
                        nop = mybir.InstNoOp(name=f"I-wsplit-{nc.next_id()}",
                                             ins=[], outs=[])
                        nop.engine = ins.engine
                        nop.sync_info = mybir.SyncInfo(on_wait=[w], on_update=[])
                        new.append(nop)
                        n_split += 1
                    ins.sync_info = mybir.SyncInfo(on_wait=keep,
                                                   on_update=list(si.on_update))
                    changed = True
                new.append(ins)
            if changed:
                il[:] = new
    return n_split


def _build_nc(split_waits=True):
    import concourse.bass as bass
    import concourse.tile as tile
    import concourse.mybir as mybir
    from contextlib import ExitStack

    f32 = mybir.dt.float32
    bf16 = mybir.dt.bfloat16
    mdt = f32 if MM_DT == "f32" else bf16
    AF = mybir.ActivationFunctionType
    ALU = mybir.AluOpType

    nc = bass.Bass()

    # ---- DRAM parameters (per core) ----
    h_d = nc.declare_dram_parameter("h_c", [NODES, IN_NF], f32, isOutput=False)
    w_in_d = nc.declare_dram_parameter("w_in", [IN_NF, H], mdt, isOutput=False)
    b_in_d = nc.declare_dram_parameter("b_in", [H], f32, isOutput=False)
    w_out_d = nc.declare_dram_parameter("w_out", [H, OUT_NF], f32, isOutput=False)
    b_out_d = nc.declare_dram_parameter("b_out", [OUT_NF], f32, isOutput=False)
    we1_d = nc.declare_dram_parameter("we1", [L, 2 * H, EH], mdt, isOutput=False)
    be1_d = nc.declare_dram_parameter("be1", [L, EH], f32, isOutput=False)
    we2_d = nc.declare_dram_parameter("we2", [L, EH, EH], mdt, isOutput=False)
    be2_d = nc.declare_dram_parameter("be2", [L, EH], f32, isOutput=False)
    wn1_d = nc.declare_dram_parameter("wn1", [L, H + EH, H], mdt, isOutput=False)
    bn1_d = nc.declare_dram_parameter("bn1", [L, H], f32, isOutput=False)
    wn1n_d = nc.declare_dram_parameter("wn1n", [L, EH, H], mdt, isOutput=False)
    wn2_d = nc.declare_dram_parameter("wn2", [L, H, H], mdt, isOutput=False)
    bn2_d = nc.declare_dram_parameter("bn2", [L, H], f32, isOutput=False)
    ident_d = nc.declare_dram_parameter("ident", [128, 128], mdt, isOutput=False)
    wab0_d = nc.declare_dram_parameter("wab0", [IN_NF, 2 * EH], mdt, isOutput=False)
    cab0_d = nc.declare_dram_parameter("cab0", [EH, 2], f32, isOutput=False)
    identf_d = nc.declare_dram_parameter("identf", [128, 128], f32, isOutput=False)
    ones_d = nc.declare_dram_parameter("ones_r", [1, 128], f32, isOutput=False)
    out_d = nc.declare_dram_parameter("out_c", [NODES, OUT_NF], f32, isOutput=True)

    with tile.TileContext(nc) as tc, ExitStack() as ctx:
        consts = ctx.enter_context(tc.tile_pool(name="consts", bufs=1))
        work = ctx.enter_context(tc.tile_pool(name="work", bufs=2))
        zpool = ctx.enter_context(tc.tile_pool(name="zp", bufs=1))
        xpool = ctx.enter_context(tc.tile_pool(name="xp", bufs=2))
        psum = ctx.enter_context(tc.tile_pool(name="ps", bufs=1, space="PSUM"))

        dma = nc.sync.dma_start

        # warm the ACT Silu table immediately (zero-dependency dummy op)
        warm = work.tile([1, 2], f32, tag="warm", name="warm")
        nc.vector.memset(warm[0:1, 0:1], 0.0)
        nc.scalar.activation(warm[0:1, 1:2], warm[0:1, 0:1], AF.Silu)

        # ---- input loads (h first: it heads the critical path) ----
        hn = work.tile([128, GPC * IN_NF], f32, tag="hn", name="hn")
        dma(out=hn[:].rearrange("p (t f) -> p t f", t=GPC),
            in_=h_d.rearrange("(t p) f -> p t f", p=128))
        hns = [hn[:, nb * IN_NF:(nb + 1) * IN_NF] for nb in range(GPC)]

        # ---- constant loads ----
        identf_sb = consts.tile([128, 128], f32, tag="identf", name="identf_sb")
        dma(out=identf_sb[:], in_=identf_d[:])
        ident_sb = consts.tile([128, 128], mdt, tag="ident", name="ident_sb")
        dma(out=ident_sb[:], in_=ident_d[:])
        wab0_sb = consts.tile([IN_NF, 2 * EH], mdt, tag="wab0", name="wab0_sb")
        dma(out=wab0_sb[:], in_=wab0_d[:])
        cab0_sb = consts.tile([EH, 2], f32, tag="cab0", name="cab0_sb")
        dma(out=cab0_sb[:], in_=cab0_d[:])

        w_in_sb = consts.tile([IN_NF, H], mdt, tag="w_in", name="w_in_sb")
        dma(out=w_in_sb[:], in_=w_in_d[:])
        b_in_sb = consts.tile([128, HC], f32, tag="b_in", name="b_in_sb")
        dma(out=b_in_sb[:], in_=b_in_d.rearrange("(m p) -> p m", p=128))

        # ---- input embedding: x_T[m] = (h @ w_in + b_in)^T ----
        hT = work.tile([IN_NF, NODES], mdt, tag="hT", name="hT")
        for nb in range(NODES // 128):
            hTp = psum.tile([IN_NF, 128], f32, tag=f"mp{nb % 2}", bufs=1,
                            name=f"hTp_{nb}")
            nc.tensor.transpose(hTp[:], hns[nb], identf_sb[:])
            nc.vector.tensor_copy(hT[:, nb * 128:(nb + 1) * 128], hTp[:])

        x_T = [xpool.tile([128, NODES], f32, tag=f"x{m}", name=f"x0_{m}")
               for m in range(HC)]
        xb = [xpool.tile([128, NODES], mdt, tag=f"xb{m}", name=f"xb0_{m}")
              for m in range(HC)]

        def x_embed(g):
            gb = slice(g * N, (g + 1) * N)
            for m in range(HC):
                xp_ = psum.tile([128, N], f32, tag=f"mp{m % 2}", bufs=1,
                                name=f"xemb_{g}_{m}")
                nc.tensor.matmul(xp_[:], lhsT=w_in_sb[:, m * 128:(m + 1) * 128],
                                 rhs=hT[:, gb], start=True, stop=True)
                nc.vector.tensor_scalar_add(x_T[m][:, gb], xp_[:], b_in_sb[:, m:m + 1])
                nc.vector.tensor_scalar_add(xb[m][:, gb], xp_[:], b_in_sb[:, m:m + 1])

        # ---- weight loads ----
        we1_sb, we2_sb, wn1_sb, wn2_sb = [], [], [], []
        be1_sb = consts.tile([EH, L], f32, tag="be1", name="be1_sb")
        dma(out=be1_sb[:], in_=be1_d.rearrange("l p -> p l"))
        be2_sb = consts.tile([EH, L], f32, tag="be2", name="be2_sb")
        bn1_sb = consts.tile([128, L * HC], f32, tag="bn1", name="bn1_sb")
        bn2_sb = consts.tile([128, L * HC], f32, tag="bn2", name="bn2_sb")
        for l in range(L):
            if l == 0:
                we1_sb.append(None)
            else:
                t1 = []
                for j in range(4):
                    t = consts.tile([128, EH], mdt, tag=f"we1_{l}_{j}",
                                    name=f"we1_{l}_{j}")
                    dma(out=t[:], in_=we1_d[l, j * 128:(j + 1) * 128, :])
                    t1.append(t[:])
                we1_sb.append(t1)
            t = consts.tile([EH, EH], mdt, tag=f"we2_{l}", name=f"we2_{l}")
            dma(out=t[:], in_=we2_d[l])
            we2_sb.append(t)
            tn = []
            for k in range(3):
                t = consts.tile([128, H], mdt, tag=f"wn1_{l}_{k}", name=f"wn1_{l}_{k}")
                dma(out=t[:], in_=wn1_d[l, k * 128:(k + 1) * 128, :])
                tn.append(t)
            wn1_sb.append(tn)
            t = consts.tile([EH, H], mdt, tag=f"wn1n_{l}", name=f"wn1n_{l}")
            dma(out=t[:], in_=wn1n_d[l])
            wn1_sb[l].append(t)
            tn = []
            for k in range(2):
                t = consts.tile([128, H], mdt, tag=f"wn2_{l}_{k}", name=f"wn2_{l}_{k}")
                dma(out=t[:], in_=wn2_d[l, k * 128:(k + 1) * 128, :])
                tn.append(t)
            wn2_sb.append(tn)
            if l == 0:
                dma(out=be2_sb[:], in_=be2_d.rearrange("l p -> p l"))
                dma(out=bn1_sb[:],
                    in_=bn1_d.rearrange("l (m p) -> p (l m)", p=128))
                dma(out=bn2_sb[:],
                    in_=bn2_d.rearrange("l (m p) -> p (l m)", p=128))
        w_out_sb = []
        for k in range(HC):
            t = consts.tile([128, OUT_NF], f32, tag=f"w_out_{k}", name=f"w_out_{k}")
            dma(out=t[:], in_=w_out_d[k * 128:(k + 1) * 128, :])
            w_out_sb.append(t)
        b_out_sb = consts.tile([1, OUT_NF], f32, tag="b_out", name="b_out_sb")
        dma(out=b_out_sb[:], in_=b_out_d[:].unsqueeze(0))
        ones_sb = consts.tile([1, 128], f32, tag="ones", name="ones_sb")
        dma(out=ones_sb[:], in_=ones_d[:])

        st = {"mp": 0, "chunk": 0, ("x", 0): (x_T, xb)}

        def mp_tag():
            st["mp"] ^= 1
            return f"mp{st['mp']}"

        def prep(l, g):
            """a/b projections for (layer l, graph g) -> aT/bT bf16 in SBUF."""
            aT = work.tile([EH, N], mdt, tag=f"aT{g}", name=f"aT_{l}_{g}")
            bT = work.tile([EH, N], mdt, tag=f"bT{g}", name=f"bT_{l}_{g}")
            st[("ab", l, g)] = (aT, bT)
            gb = slice(g * N, (g + 1) * N)
            if l == 0:
                # layer 0 shortcut: a|b = hT @ (w_in@we1) + (b_in@we1 + be1),
                # host-precomputed -> skips the embedding chain at startup
                pb = psum.tile([EH, 2 * N], f32, tag=mp_tag(), bufs=1,
                               name=f"prep0_{g}")
                nc.tensor.matmul(pb[:, 0:N], lhsT=wab0_sb[:, 0:EH],
                                 rhs=hT[:, gb], start=True, stop=True)
                nc.tensor.matmul(pb[:, N:2 * N], lhsT=wab0_sb[:, EH:2 * EH],
                                 rhs=hT[:, gb], start=True, stop=True)
                nc.vector.tensor_scalar_add(aT[:], pb[:, 0:N], cab0_sb[:, 0:1])
                nc.vector.tensor_scalar_add(bT[:], pb[:, N:2 * N], cab0_sb[:, 1:2])
                return
            xb_cur = st[("x", l)][1]
            blob = st["blob"]
            ap_ = blob[:, 4 * N:5 * N]
            for k in range(HC):
                nc.tensor.matmul(ap_, lhsT=we1_sb[l][k],
                                 rhs=xb_cur[k][:, gb],
                                 start=(k == 0), stop=(k == HC - 1))
            nc.vector.tensor_scalar_add(aT[:], ap_, be1_sb[:, l:l + 1])
            bp_ = blob[:, 5 * N:6 * N]
            for k in range(HC):
                nc.tensor.matmul(bp_, lhsT=we1_sb[l][HC + k],
                                 rhs=xb_cur[k][:, gb],
                                 start=(k == 0), stop=(k == HC - 1))
            nc.vector.tensor_copy(bT[:], bp_)

        def zbuild(l, g, c, p):
            """z1 piece p = a_i + b_j built in SBUF on DVE."""
            if p == 0:
                k = st["chunk"] % 2
                st["chunk"] += 1
                st[("z1", l, g, c)] = zpool.tile(
                    [EH, CHUNK], mdt, tag=f"zc{k}", name=f"z1_{l}_{g}_{c}")
            z1 = st[("z1", l, g, c)]
            aT, bT = st[("ab", l, g)]
            R = SLAB_RECV
            s0 = sum(ZPIECES[:p])
            ns = ZPIECES[p]
            eng = nc.gpsimd if ZENG[p] == "gps" else nc.vector
            # receiver-major layout: col = r*N + j
            nr = ns * R
            rl = (c * NSLAB + s0) * R
            eng.tensor_tensor(
                z1[:, s0 * SLAB:(s0 + ns) * SLAB]
                    .rearrange("p (r j) -> p r j", j=N),
                bT[:].unsqueeze(1).broadcast_to([EH, nr, N]),
                aT[:, rl:rl + nr].unsqueeze(2).broadcast_to([EH, nr, N]),
                op=ALU.add)

        def silu1(l, g, c, p):
            z1 = st[("z1", l, g, c)]
            if p == 0:
                st[("m1", l, g, c)] = zpool.tile(
                    [EH, CHUNK], mdt, tag=f"m1_{(g * NCHUNK + c) % 2}",
                    name=f"m1_{l}_{g}_{c}")
            m1 = st[("m1", l, g, c)]
            lo = sum(ZPIECES[:p]) * SLAB
            hi = lo + ZPIECES[p] * SLAB
            nc.scalar.activation(m1[:, lo:hi], z1[:, lo:hi], AF.Silu)

        def silu1h(l, g, c, p):
            """silu1 as half-chunk instructions (block-tail placement)."""
            z1 = st[("z1", l, g, c)]
            if p == 0:
                st[("m1", l, g, c)] = zpool.tile(
                    [EH, CHUNK], mdt, tag=f"m1_{(g * NCHUNK + c) % 2}",
                    name=f"m1_{l}_{g}_{c}")
            m1 = st[("m1", l, g, c)]
            lo = p * (CHUNK // 2)
            hi = lo + CHUNK // 2
            nc.scalar.activation(m1[:, lo:hi], z1[:, lo:hi], AF.Silu)

        def slab_act(l, g, c, s):
            """mm2 -> silu2 for one 2048-col slab."""
            if ("agg", l) not in st:
                st[("agg", l)] = work.tile([EH, NODES], mdt, tag="agg",
                                           name=f"agg_{l}")
                st[("diag", l)] = work.tile([EH, NODES], mdt, tag="diag",
                                            name=f"diag_{l}")
            m1 = st[("m1", l, g, c)]
            mp_ = psum.tile([EH, SLAB], f32, tag=mp_tag(), bufs=1,
                            name=f"mp_{l}_{g}_{c}_{s}")
            for q in range(SLAB // MMQ):
                nc.tensor.matmul(mp_[:, q * MMQ:(q + 1) * MMQ],
                                 lhsT=we2_sb[l][:],
                                 rhs=m1[:, s * SLAB + q * MMQ:
                                        s * SLAB + (q + 1) * MMQ],
                                 start=True, stop=True)
            m2 = zpool.tile([EH, SLAB], mdt, tag=f"m2_{s}",
                            name=f"m2_{l}_{g}_{c}_{s}")
            st[("m2", g, c, s)] = m2
            nc.scalar.activation(m2[:], mp_[:], AF.Silu, bias=be2_sb[:, l:l + 1])

        def slab_dve(l, g, c, s):
            """Diag extract (GPSIMD) + DVE 2x folds from SBUF + reduce.
            Slab-major cols (j, r) make the j-halves contiguous, so the
            folds are packed bf16 tensor_tensor adds (2x mode), and the
            PSUM tags cycle on [mm2 -> silu2] alone (no serial loop)."""
            aggT, diagT = st[("agg", l)], st[("diag", l)]
            m2 = st[("m2", g, c, s)]
            R = SLAB_RECV
            r0 = c * CHUNK_RECV + s * R            # receiver offset in graph
            # receiver-major: receiver r0+r's own column is r*N + r0 + r
            diag_ap = bass.AP(
                tensor=m2.tensor, offset=m2.offset + r0,
                ap=[m2.ap[0], [N + 1, R]])
            nc.vector.tensor_copy(
                diagT[:, g * N + r0: g * N + r0 + R], diag_ap)
            m2v = m2[:].rearrange("p (r j) -> p r j", j=N)
            f1 = zpool.tile([EH, SLAB // 2], mdt, tag=f"f1_{s % 2}",
                            name=f"f1_{l}_{g}_{c}_{s}")
            f1v = f1[:].rearrange("p (r j) -> p r j", j=N // 2)
            nc.vector.tensor_tensor(f1v, m2v[:, :, 0:N // 2],
                                    m2v[:, :, N // 2:N], op=ALU.add)
            f2 = zpool.tile([EH, SLAB // 4], mdt, tag=f"f2_{s % 2}",
                            name=f"f2_{l}_{g}_{c}_{s}")
            f2v = f2[:].rearrange("p (r j) -> p r j", j=N // 4)
            nc.vector.tensor_tensor(f2v, f1v[:, :, 0:N // 4],
                                    f1v[:, :, N // 4:N // 2], op=ALU.add)
            with nc.allow_low_precision(reason="bf16 agg"):
                nc.vector.tensor_reduce(
                    aggT[:, g * N + r0: g * N + r0 + R],
                    f2v, axis=mybir.AxisListType.X, op=ALU.add)

        def node(l, g, blob):
            """node MLP + residual for (layer l, graph g); diag correction is
            fused into the up-matmul via negated agg-row weights (wn1n)."""
            if ("u1", l) not in st:
                st[("u1", l)] = [work.tile([128, NODES], mdt, tag=f"u1_{m}",
                                           name=f"u1_{l}_{m}")
                                 for m in range(HC)]
                st[("x", l + 1)] = (
                    [xpool.tile([128, NODES], f32, tag=f"x{m}",
                                name=f"x{l + 1}_{m}") for m in range(HC)],
                    [xpool.tile([128, NODES], mdt, tag=f"xb{m}",
                                name=f"xb{l + 1}_{m}") for m in range(HC)])
            u1 = st[("u1", l)]
            aggT, diagT = st[("agg", l)], st[("diag", l)]
            x_cur, xb_cur = st[("x", l)]
            x_new, xb_new = st[("x", l + 1)]
            gb = slice(g * N, (g + 1) * N)
            for m in range(HC):
                up = blob[:, m * N:(m + 1) * N]
                nc.tensor.matmul(up, lhsT=wn1_sb[l][0][:, m * 128:(m + 1) * 128],
                                 rhs=xb_cur[0][:, gb], start=True, stop=False)
                nc.tensor.matmul(up, lhsT=wn1_sb[l][1][:, m * 128:(m + 1) * 128],
                                 rhs=xb_cur[1][:, gb], start=False, stop=False)
                nc.tensor.matmul(up, lhsT=wn1_sb[l][2][:, m * 128:(m + 1) * 128],
                                 rhs=aggT[:, gb], start=False, stop=False)
                nc.tensor.matmul(up, lhsT=wn1_sb[l][3][:, m * 128:(m + 1) * 128],
                                 rhs=diagT[:, gb], start=False, stop=True)
                nc.scalar.activation(u1[m][:, gb], up, AF.Silu,
                                     bias=bn1_sb[:, l * HC + m: l * HC + m + 1])
            for m in range(HC):
                u2p = blob[:, (2 + m) * N:(3 + m) * N]
                nc.tensor.matmul(u2p, lhsT=wn2_sb[l][0][:, m * 128:(m + 1) * 128],
                                 rhs=u1[0][:, gb], start=True, stop=False)
                nc.tensor.matmul(u2p, lhsT=wn2_sb[l][1][:, m * 128:(m + 1) * 128],
                                 rhs=u1[1][:, gb], start=False, stop=True)
                if l + 1 < L:
                    nc.vector.scalar_tensor_tensor(
                        xb_new[m][:, gb], u2p,
                        bn2_sb[:, l * HC + m: l * HC + m + 1], x_cur[m][:, gb],
                        op0=ALU.add, op1=ALU.add)
                nc.vector.scalar_tensor_tensor(
                    x_new[m][:, gb], u2p,
                    bn2_sb[:, l * HC + m: l * HC + m + 1], x_cur[m][:, gb],
                    op0=ALU.add, op1=ALU.add)

        def out(g):
            x_fin = st[("x", L)][0]
            blob = st["blob"]
            op_ = blob[:, 6 * N:6 * N + OUT_NF]
            nc.tensor.matmul(op_, lhsT=x_fin[0][:, g * 128:(g + 1) * 128],
                             rhs=w_out_sb[0][:], start=True, stop=False)
            nc.tensor.matmul(op_, lhsT=x_fin[1][:, g * 128:(g + 1) * 128],
                             rhs=w_out_sb[1][:], start=False, stop=False)
            nc.tensor.matmul(op_, lhsT=ones_sb[0:1, 0:128], rhs=b_out_sb[0:1, :],
                             start=False, stop=True)
            ob = work.tile([128, OUT_NF], f32, tag="ob", name=f"ob_{g}")
            nc.vector.tensor_copy(ob[:], op_)
            dma(out=out_d[g * 128:(g + 1) * 128, :], in_=ob[:])

        # ---- emission schedule ----
        # Per block: the current chunk's silu2 slabs run front-loaded (s0,s1
        # first so PE/PSUM stay hot), the boundary chain (node -> prep ->
        # zbuild) for the NEXT chunk runs through a single parallel-slice
        # PSUM blob, and the next chunk's silu1 halves run at the block tail
        # where zbuild has already finished with large slack.
        def new_blob(name):
            st["blob"] = psum.tile([128, SLAB], f32, tag=mp_tag(), bufs=1,
                                   name=name)

        seq = [(l, g) for l in range(L) for g in range(GPC)]
        events = [("prep0",), ("zb+", 0, 0)]
        for idx, (l, g) in enumerate(seq):
            acts = [("slab_act", l, g, 0, t) for t in range(NSLAB)]
            dves = [("slab_dve", l, g, 0, t) for t in range(NSLAB)]
            blk = [acts[0], acts[1]]
            s1 = []
            if idx + 1 < len(seq):
                ln, gn = seq[idx + 1]
                blk.append(("bound", ln, gn))
                blk.append(("zb+", ln, gn))
                s1 = [("silu1", ln, gn, 0, p) for p in range(2)]
            else:
                blk += [("bound", L, 0), ("outg", 0)]
            if idx == 0:
                blk += [("xemb", 0), ("xemb", 1)]
            for t in range(2, NSLAB):
                blk += [acts[t], dves[t - 2]]
            blk += [dves[NSLAB - 2], dves[NSLAB - 1]] + s1
            events += blk
        events += [("bound", L, 1), ("outg", 1)]

        nzp = len(ZPIECES)
        for ev in events:
            if ev[0] == "prep0":
                new_blob("blob_p0")
                prep(0, 0)
            elif ev[0] == "bound":
                new_blob(f"blob_{ev[1]}_{ev[2]}")
                if ev[1] > 0:
                    node(ev[1] - 1, ev[2], st["blob"])
                if ev[1] < L:
                    prep(ev[1], ev[2])
            elif ev[0] == "outg":
                out(ev[1])
            elif ev[0] == "zb+":
                for p in range(nzp):
                    zbuild(ev[1], ev[2], 0, p)
                if (ev[1], ev[2]) == (0, 0):
                    for p in range(2):
                        silu1h(0, 0, 0, p)
            elif ev[0] == "silu1":
                silu1h(ev[1], ev[2], ev[3], ev[4])
            elif ev[0] == "slab_act":
                slab_act(ev[1], ev[2], ev[3], ev[4])
            elif ev[0] == "slab_dve":
                slab_dve(ev[1], ev[2], ev[3], ev[4])
            elif ev[0] == "xemb":
                x_embed(ev[1])

    if split_waits:
        _split_excess_waits(nc, mybir)
    return nc


def _get_nc():
    if "nc" not in _CACHE:
        _CACHE["nc"] = _build_nc()
    return _CACHE["nc"]


def _to_mdt(a):
    if MM_DT == "bf16":
        import ml_dtypes
        return np.asarray(a, dtype=np.float32).astype(ml_dtypes.bfloat16)
    return np.asarray(a, dtype=np.float32)


def _run_on_hw(inputs, **spmd_kwargs):
    """Shard, run on the 8 NeuronCores, gather. Returns (out, BassKernelResults)."""
    from concourse.bass_utils import run_bass_kernel_spmd

    f = np.float32
    h = np.ascontiguousarray(np.asarray(inputs["h"], dtype=f))
    ws = {k: np.ascontiguousarray(np.asarray(inputs[k], dtype=f))
          for k in ("w_in", "b_in", "w_out", "b_out", "we1", "be1", "we2",
                    "be2", "wn1", "bn1", "wn2", "bn2")}
    nc = _get_nc()
    f64 = np.float64
    we1f = ws["we1"][0].astype(f64)
    wab0 = np.concatenate([ws["w_in"].astype(f64) @ we1f[:H],
                           ws["w_in"].astype(f64) @ we1f[H:]], axis=1)
    cab0 = np.stack([ws["b_in"].astype(f64) @ we1f[:H] + ws["be1"][0],
                     ws["b_in"].astype(f64) @ we1f[H:]], axis=1)
    base = {
        "wab0": _to_mdt(wab0.astype(f)), "cab0": np.ascontiguousarray(cab0.astype(f)),
        "w_in": _to_mdt(ws["w_in"]), "b_in": ws["b_in"],
        "w_out": ws["w_out"], "b_out": ws["b_out"],
        "we1": _to_mdt(ws["we1"]), "be1": ws["be1"],
        "we2": _to_mdt(ws["we2"]), "be2": ws["be2"],
        "wn1": _to_mdt(ws["wn1"]), "bn1": ws["bn1"],
        "wn1n": _to_mdt(-ws["wn1"][:, H:H + EH, :]),
        "wn2": _to_mdt(ws["wn2"]), "bn2": ws["bn2"],
        "ident": _to_mdt(np.eye(128, dtype=f)),
        "identf": np.eye(128, dtype=f),
        "ones_r": np.ones((1, 128), dtype=f),
    }
    in_maps = []
    for c in range(NCORES):
        m = dict(base)
        m["h_c"] = np.ascontiguousarray(
            h[c * GPC:(c + 1) * GPC].reshape(NODES, IN_NF))
        in_maps.append(m)

    res = run_bass_kernel_spmd(nc, in_maps, list(range(NCORES)), **spmd_kwargs)
    out = np.concatenate([np.asarray(res.results[i]["out_c"], dtype=f)
                          for i in range(NCORES)], axis=0)
    return out, res


def kernel(**inputs):
    h = np.asarray(inputs["h"])
    rows, cols = inputs["rows"], inputs["cols"]
    if h.shape != (BS, N, IN_NF) or not _edges_match(rows, cols):
        ws = {k: np.asarray(inputs[k], dtype=np.float32)
              for k in ("w_in", "b_in", "w_out", "b_out", "we1", "be1", "we2",
                        "be2", "wn1", "bn1", "wn2", "bn2")}
        return _numpy_reference(np.asarray(h, np.float32), rows, cols, **ws)
    out, _ = _run_on_hw(inputs)
    return out
